# revision 3
# baseline (speedup 1.0000x reference)
"""BalanceLoss (BCE + OHEM top-k negatives) on 8 trn2 NeuronCores — v7.

Math (gt, mask in {0,1}, pred in (0,1)):
    mask * ln(select(gt, pred, 1-pred)) == ln(1 + h*d)   pointwise, with
    d = gt - pred,  h = (1 - 2*gt)*mask   (h*d = -mask*|gt-pred|).
Device sums:  sc = sum ln(1+h*d)             (Act Ln accumulators)
              e1 = sum h = sn - sw           (DVE AMR accumulators)
              sm = sum mask = sn + sw        (PE ones-matmuls into PSUM)
Host: sw = (sm-e1)/2, sn = (sm+e1)/2; OHEM top-k == full negative sum when
min(sn, 3*sw) == sn (true for this distribution; exact host fallback kept).

Scheduling: fully explicit per-engine instruction streams (EMIT list).
gt/mask tiles 1-7 arrive as fp8 casting DMAs on the gpsimd queue ({0,1}
exact, 1-byte transfer charge); tile 0 f32 via the Act queue; pred f32
via SP. DMAs occupy their issuing engine for the transfer in this cost
model, so bytes and compute are budgeted jointly per engine.
"""

import os
import sys

import numpy as np

FULL_SHAPE = (32, 1, 640, 640)
TOT = 32 * 640 * 640
N_CORES = 8
PER_CORE = TOT // N_CORES     # 1_638_400
P = 128
W = PER_CORE // P             # 12_800
NT = 8
F = W // NT                   # 1600

# EMIT: ordered instruction stream; engines execute their own subsequences
# in this order. Ops:
#   ("sp", tens, c0, c1)    SP-queue f32 DMA
#   ("act", tens, c0, c1)   Act-queue f32 DMA
#   ("pool", tens, c0, c1)  Pool-queue fp8 casting DMA
#   ("d", t, eng) ("q", t, eng)  tensor_tensor on "dve"/"pool"
#   ("h", t)                AMR on DVE
#   ("ln", t)               Act Ln + accum; also emits PE matmuls for t
EMIT = [
    ("pool", "gt", 1600, 3200), ("pool", "mask", 1600, 3200),
    ("sp", "pred", 1600, 3200), ("sp", "pred", 3200, 4800),
    ("sp", "pred", 4800, 6400), ("sp", "pred", 6400, 8000),
    ("sp", "pred", 0, 1600), ("sp", "pred", 8000, 9600),
    ("sp", "pred", 9600, 11200), ("sp", "pred", 11200, 12800),
    ("act", "mask", 0, 1600), ("act", "gt", 0, 1600),
    ("warm",),
    ("h", 1600, 3200, 1),
    ("pool", "gt", 3200, 6400), ("pool", "mask", 3200, 6400),
    ("d", 1600, 3200, "pool"),
    ("q", 1600, 3200, "dve"), ("mm", 1600, 3200),
    ("h", 3200, 4800, 2),
    ("pool", "gt", 6400, 9600), ("pool", "mask", 6400, 9600),
    ("d", 3200, 4800, "pool"),
    ("ln", 1600, 3200, 1),
    ("h", 4800, 6400, 3),
    ("q", 3200, 4800, "pool"), ("mm", 3200, 4800),
    ("ln", 3200, 4800, 2),
    ("pool", "gt", 9600, 11200), ("pool", "mask", 9600, 11200),
    ("d", 4800, 6400, "pool"),
    ("q", 4800, 6400, "dve"), ("mm", 4800, 6400),
    ("h", 6400, 8000, 4),
    ("pool", "gt", 11200, 12800), ("pool", "mask", 11200, 12800),
    ("ln", 4800, 6400, 3),
    ("d", 6400, 8000, "pool"),
    ("h", 0, 1600, 0),
    ("q", 6400, 8000, "pool"), ("mm", 6400, 8000),
    ("ln", 6400, 8000, 4),
    ("d", 0, 1600, "pool"),
    ("q", 0, 1600, "dve"), ("mm", 0, 1600),
    ("h", 8000, 9600, 5),
    ("d", 8000, 9600, "pool"),
    ("ln", 0, 1600, 0),
    ("h", 9600, 11200, 6), ("h", 11200, 12800, 7),
    ("q", 8000, 9600, "pool"), ("mm", 8000, 9600),
    ("ln", 8000, 9600, 5),
    ("d", 9600, 11200, "pool"),
    ("q", 9600, 11200, "dve"), ("mm", 9600, 11200),
    ("ln", 9600, 11200, 6),
    ("d", 11200, 12800, "pool"),
    ("q", 11200, 12800, "dve"), ("mm", 11200, 12800),
    ("ln", 11200, 12800, 7),
]

MMCHUNK = 320
NEG_RATIO = 3.0
EPS = 1e-6
F8_TILES = (1, 2, 3, 4, 5, 6, 7)

_CONCOURSE_PATHS = ("/opt/trn_rl_repo", "/root/.axon_site/_ro/trn_rl_repo")


def _ensure_concourse():
    try:
        import concourse.bass  # noqa: F401
    except ImportError:
        for p in _CONCOURSE_PATHS:
            if os.path.isdir(p) and p not in sys.path:
                sys.path.insert(0, p)
        import concourse.bass  # noqa: F401


_NC_CACHE = {}


def _build_nc(reps=1):
    if reps in _NC_CACHE:
        return _NC_CACHE[reps]
    _ensure_concourse()
    import concourse.bacc as bacc
    import concourse.mybir as mybir
    import concourse.tile as tile

    f32 = mybir.dt.float32
    f8 = mybir.dt.float8e4
    bf16 = mybir.dt.bfloat16
    ActF = mybir.ActivationFunctionType
    Alu = mybir.AluOpType

    nc = bacc.Bacc(None, target_bir_lowering=False)
    predD = nc.declare_dram_parameter("pred", [P, W], f32, isOutput=False)
    gtD = nc.declare_dram_parameter("gt", [P, W], f32, isOutput=False)
    maskD = nc.declare_dram_parameter("mask", [P, W], f32, isOutput=False)
    outD = nc.declare_dram_parameter("stats", [P, 2 * NT + 2], f32, isOutput=True)
    msumD = nc.declare_dram_parameter("msum", [1, MMCHUNK], f32, isOutput=True)
    dram = {"pred": predD, "gt": gtD, "mask": maskD}
    qeng = {"sp": "sync", "act": "scalar", "pool": "gpsimd"}

    n_mms = W // MMCHUNK

    with tile.TileContext(nc) as tc:
        with (
            tc.tile_pool(name="io", bufs=1) as io_pool,
            tc.tile_pool(name="tmp", bufs=3) as tmp_pool,
            tc.tile_pool(name="accp", bufs=1) as acc_pool,
            tc.tile_pool(name="ps", bufs=1, space="PSUM") as ps_pool,
        ):
            acc_h = acc_pool.tile([P, NT + 1], f32)
            nc.vector.memset(acc_h[:], 0.0)
            acc_ln = acc_pool.tile([P, NT + 1], f32)
            nc.vector.memset(acc_ln[:], 0.0)
            ones_f = acc_pool.tile([P, 1], f32)
            nc.gpsimd.memset(ones_f[:], 1.0)
            ones_8 = acc_pool.tile([P, 1], f8)
            nc.gpsimd.memset(ones_8[:], 1.0)
            psum = ps_pool.tile([1, MMCHUNK], f32)
            warm = acc_pool.tile([1, 1], f32)
            nc.gpsimd.memset(warm[:], 0.0)

            for rep in range(reps):
                views = {"pred": [], "gt": [], "mask": []}
                tiles_d = {}
                tiles_h = {}
                tiles_q = {}
                mmi = [0]

                def cview(tens, lo, hi):
                    for c0, c1, b in views[tens]:
                        if c0 <= lo and hi <= c1:
                            return b[:, lo - c0:hi - c0]
                    raise AssertionError(f"no chunk for {tens} [{lo},{hi})")

                def bview(tbl, lo, hi):
                    for (c0, c1), b in tbl.items():
                        if c0 <= lo and hi <= c1:
                            return b[:, lo - c0:hi - c0]
                    raise AssertionError(f"no tile buf [{lo},{hi})")

                for oi, op in enumerate(EMIT):
                    kind = op[0]
                    if kind in ("sp", "act", "pool"):
                        _, tens, c0, c1 = op
                        dt = f8 if kind == "pool" else f32
                        b = io_pool.tile([P, c1 - c0], dt,
                                         tag=f"io{oi}_{rep}")
                        getattr(nc, qeng[kind]).dma_start(
                            b[:], dram[tens][:, c0:c1])
                        views[tens].append((c0, c1, b))
                    elif kind == "d":
                        _, c0, c1, eng = op
                        d = tmp_pool.tile([P, c1 - c0], f32, tag=f"d{c1-c0}")
                        e = nc.vector if eng == "dve" else nc.gpsimd
                        e.tensor_tensor(d[:], cview("gt", c0, c1),
                                        cview("pred", c0, c1), Alu.subtract)
                        tiles_d[(c0, c1)] = d
                    elif kind == "h":
                        _, c0, c1, col = op
                        h = tmp_pool.tile([P, c1 - c0], bf16, tag=f"h{c1-c0}")
                        nc.vector.affine_mul_reduce(
                            out=h[:], accum_out=acc_h[:, col:col + 1],
                            in0=cview("gt", c0, c1), in1=cview("mask", c0, c1),
                            scale=-2.0, bias=1.0)
                        tiles_h[(c0, c1)] = h
                    elif kind == "q":
                        _, c0, c1, eng = op
                        q = tmp_pool.tile([P, c1 - c0], f32, tag=f"q{c1-c0}")
                        e = nc.vector if eng == "dve" else nc.gpsimd
                        e.tensor_tensor(q[:], bview(tiles_h, c0, c1),
                                        bview(tiles_d, c0, c1), Alu.mult)
                        tiles_q[(c0, c1)] = q
                    elif kind == "warm":
                        wj = acc_pool.tile([1, 1], f32, tag=f"wj_{rep}")
                        nc.scalar.activation(wj[0:1, 0:1], warm[0:1, 0:1],
                                             ActF.Ln, bias=1.0, scale=1.0)
                    elif kind == "ln":
                        _, c0, c1, col = op
                        nc.scalar.activation(bview(tiles_d, c0, c1),
                                             bview(tiles_q, c0, c1),
                                             ActF.Ln, bias=1.0, scale=1.0,
                                             accum_out=acc_ln[:, col:col + 1])
                    elif kind == "mm":
                        _, c0, c1 = op
                        f8r = c0 >= 1600
                        ones = ones_8 if f8r else ones_f
                        for c in range(c0, c1, MMCHUNK):
                            nc.tensor.matmul(
                                psum[0:1, :], ones[:, 0:1],
                                cview("mask", c, c + MMCHUNK),
                                start=(mmi[0] == 0),
                                stop=(mmi[0] == n_mms - 1),
                                skip_group_check=True)
                            mmi[0] += 1
                msb = acc_pool.tile([1, MMCHUNK], f32, tag="msb")
                nc.scalar.activation(msb[0:1, :], psum[0:1, :], ActF.Copy)
            nc.sync.dma_start(outD[:, 0:NT + 1], acc_h[:])
            nc.scalar.dma_start(outD[:, NT + 1:2 * NT + 2], acc_ln[:])
            nc.sync.dma_start(msumD[:], msb[:])
    nc.finalize()

    _NC_CACHE[reps] = nc
    return nc


def _final_scalar(e1, sm, sc, pred=None, gt=None, mask=None):
    """Host merge: e1 = sn - sw, sm = sn + sw, sc = -(pos_loss + neg_loss)."""
    sw = (sm - e1) / 2.0
    sn = (sm + e1) / 2.0
    pos_count = sw
    neg_count = min(sn, NEG_RATIO * pos_count)
    if neg_count >= sn:
        total_loss = -sc
    else:
        # exact OHEM fallback (not triggered for the shipped distribution)
        k = int(neg_count)
        p = np.asarray(pred, dtype=np.float64).ravel()
        g = np.asarray(gt, dtype=np.float64).ravel()
        m = np.asarray(mask, dtype=np.float64).ravel()
        pos_loss_sum = float(-(g * m * np.log(p)).sum())
        neg_loss = (1.0 - g) * m * (-np.log1p(-p))
        if k <= 0:
            topk_sum = 0.0
        else:
            part = np.partition(neg_loss, neg_loss.size - k)
            topk_sum = float(part[neg_loss.size - k:].sum())
        total_loss = pos_loss_sum + topk_sum
        if neg_count <= 0:
            return np.float32(pos_loss_sum / (pos_count + EPS)).reshape(())
    if neg_count > 0:
        out = total_loss / (pos_count + neg_count + EPS)
    else:
        out = total_loss / (pos_count + EPS)
    return np.asarray(out, dtype=np.float32).reshape(())


def run_device(pred, gt, mask, trace=False, reps=1, **run_kwargs):
    _ensure_concourse()
    from concourse.bass_utils import run_bass_kernel_spmd

    nc = _build_nc(reps)
    shards = []
    for a in (pred, gt, mask):
        arr = np.ascontiguousarray(np.asarray(a, dtype=np.float32)).reshape(
            N_CORES, P, W
        )
        shards.append(arr)
    in_maps = [
        {"pred": shards[0][i], "gt": shards[1][i], "mask": shards[2][i]}
        for i in range(N_CORES)
    ]
    res = run_bass_kernel_spmd(nc, in_maps, list(range(N_CORES)), trace=trace,
                               **run_kwargs)
    e1 = sc = sm = 0.0
    for r in res.results:
        stats = np.asarray(r["stats"], dtype=np.float64)
        e1 += stats[:, 0:NT + 1].sum()
        sc += stats[:, NT + 1:2 * NT + 2].sum()
        sm += np.asarray(r["msum"], dtype=np.float64).sum()
    return (e1, sm, sc), res


def kernel(pred, gt, mask):
    pred = np.asarray(pred, dtype=np.float32)
    gt = np.asarray(gt, dtype=np.float32)
    mask = np.asarray(mask, dtype=np.float32)
    if pred.shape != FULL_SHAPE:
        p64 = pred.astype(np.float64)
        g64 = gt.astype(np.float64)
        m64 = mask.astype(np.float64)
        sw = float((g64 * m64).sum())
        sn = float(((1.0 - g64) * m64).sum())
        sc = float((g64 * m64 * np.log(p64)).sum()
                   + ((1.0 - g64) * m64 * np.log1p(-p64)).sum())
        return _final_scalar(sn - sw, sn + sw, sc, pred, gt, mask)
    (e1, sm, sc), _ = run_device(pred, gt, mask)
    return _final_scalar(e1, sm, sc, pred, gt, mask)
